# revision 22
# baseline (speedup 1.0000x reference)
"""Local2d (unshared-weight conv) Bass kernel for 8 trn2 NeuronCores.

Problem: input (64,64,32,32), weight (32,32,128,64,3,3), bias (128,32,32)
-> out (64,128,32,32).  K=3, stride 1, pad 1.

Sharding: spatial over h_out — core i handles output rows 4i..4i+3 and
reads the disjoint weight slice for those rows, plus a 6-row input halo
slab.

The kernel is DMA-bound on the weight stream, so precision is chosen to
minimize bytes within the 2e-2 tolerance:
  - weights: fp8 e3m4, pre-scaled by 32 on host (keeps the randn/24
    values out of the subnormal range); 9.4MB/core.
  - input: bf16, pre-scaled by 1/32 (exact in bf16) so the matmul
    product needs no descale; 1.7MB/core.
  - output: bf16 on device, upcast to f32 on host; 2.1MB/core.
Measured end-to-end rel err ~9.5e-3 vs the 2e-2 gate.

Per output location (ho,wo) the contraction (c,ki,kj)=576 is 9 PE
matmuls of K=64 (channels) accumulating in PSUM, moving operand = input
columns [64(c), 64(b)], stationary = per-location weights [64(c), 128(o)].
"""

import numpy as np
import ml_dtypes

B, C, O, KK, H, W = 64, 64, 128, 3, 32, 32
HO = WO = 32
NCORES = 8
RPC = HO // NCORES          # output rows per core
LOCS = RPC * WO             # locations per core
G = 8                       # locations per weight-DMA group
NG = LOCS // G


def _build_bass():
    from concourse import bacc
    import concourse.mybir as mybir
    from concourse.tile import TileContext

    f32 = mybir.dt.float32
    bf16 = mybir.dt.bfloat16
    f8 = mybir.dt.float8e3
    nc = bacc.Bacc("TRN2", target_bir_lowering=False, debug=False,
                   num_devices=NCORES)

    # input slab without the zero pad columns (those are memset on-chip):
    # [64(c), 6, 32, 64(b)] bf16, values pre-scaled by 1/32.
    slab_d = nc.dram_tensor("slab", (64, RPC + 2, W, B), bf16,
                            kind="ExternalInput").ap()
    # weights pre-scaled by 32, e3m4, partition-major per group:
    # [g][c(64 part)][j, ki, kj, o] fully contiguous per-partition runs.
    wt_d = nc.dram_tensor("wt", (NG, 64, G, KK, KK, O), f8,
                          kind="ExternalInput").ap()
    bias_d = nc.dram_tensor("bias", (O, LOCS), bf16,
                            kind="ExternalInput").ap()
    out_d = nc.dram_tensor("out", (RPC, O, WO, B), bf16,
                           kind="ExternalOutput").ap()

    with TileContext(nc) as tc:
        with tc.tile_pool(name="xslab", bufs=1) as xpool, \
             tc.tile_pool(name="wpool", bufs=6) as wpool, \
             tc.tile_pool(name="bpool", bufs=1) as bpool, \
             tc.tile_pool(name="opool", bufs=4) as opool, \
             tc.tile_pool(name="psum", bufs=8, space="PSUM") as pspool:

            X = xpool.tile([64, RPC + 2, W + 2, B], bf16)
            # pad columns 0 and 33 are zeros, built on-chip
            nc.vector.memset(X[:, :, 0:1, :], 0.0)
            nc.vector.memset(X[:, :, W + 1:W + 2, :], 0.0)
            # chunked so row-0 matmuls can start long before the full slab
            # lands: rows 0-2 cols 1-4 first (enough for the first locs),
            # the rest of rows 0-2 trickled between early weight chunks, and
            # rows 3-5 staged below, just ahead of first use.
            nc.scalar.dma_start(X[:, 0:3, 1:5], slab_d[:, 0:3, 0:4])
            nc.scalar.dma_start(X[:, 0:3, 5:13], slab_d[:, 0:3, 4:12])

            bias_t = bpool.tile([128, LOCS], bf16)
            nc.gpsimd.dma_start(bias_t, bias_d)

            out_rows = {}
            for g in range(NG):
                wt = wpool.tile([64, G, KK, KK, O], f8, tag="wt")
                if g == 0:
                    # tiny first chunks so matmuls can start sooner
                    nc.sync.dma_start(wt[:, 0:1], wt_d[g, :, 0:1])
                    nc.sync.dma_start(wt[:, 1:4], wt_d[g, :, 1:4])
                    nc.sync.dma_start(wt[:, 4:G], wt_d[g, :, 4:G])
                else:
                    nc.sync.dma_start(wt, wt_d[g])
                if g == 0:
                    nc.scalar.dma_start(X[:, 0:3, 13:23], slab_d[:, 0:3, 12:22])
                elif g == 1:
                    nc.scalar.dma_start(X[:, 0:3, 23:33], slab_d[:, 0:3, 22:32])
                elif g in (3, 6, 10):
                    # stage slab rows just ahead of first use:
                    # row 3 from loc 32 (hol=1), row 4 from 64, row 5 from 96
                    r = 3 + (g > 3) + (g > 6)
                    nc.scalar.dma_start(X[:, r:r + 1, 1:33], slab_d[:, r:r + 1])

                for j in range(G):
                    loc = g * G + j
                    hol, wo = divmod(loc, WO)
                    if wo == 0:
                        out_rows[hol] = opool.tile([128, WO, B], bf16,
                                                   tag="orow",
                                                   name=f"orow{hol}")
                    orow = out_rows[hol]

                    if wo % 4 == 0:
                        ps4 = pspool.tile([128, 4, B], f32, tag="ps4",
                                          name=f"ps{loc}")
                    half = ps4[:, wo % 4, :]
                    n = 0
                    for ki in range(KK):
                        for kj in range(KK):
                            nc.tensor.matmul(half, wt[:, j, ki, kj, :],
                                             X[:, hol + ki, wo + kj, :],
                                             start=(n == 0), stop=(n == 8))
                            n += 1
                    if wo % 4 == 3:
                        nc.vector.tensor_tensor(
                            orow[:, wo - 3:wo + 1, :], ps4,
                            bias_t[:, loc - 3:loc + 1, None]
                            .to_broadcast((128, 4, B)),
                            mybir.AluOpType.add)

                    # rows 0-2: whole-row flushes on the Pool queue — their
                    # transfers slot into DMA idle gaps without ever blocking
                    # weight prefetch on the sync queue.
                    if hol != RPC - 1 and wo == WO - 1:
                        nc.gpsimd.dma_start(out_d[hol], orow)

            # last row: fine-grained flushes emitted after every weight DMA
            # so the in-order sync queue can never stall a weight load, and
            # the tail after the final matmul is one small chunk.
            last = out_rows[RPC - 1]
            for lo, hi in ((0, 8), (8, 16), (16, 24), (24, 28), (28, 32)):
                nc.sync.dma_start(out_d[RPC - 1, :, lo:hi, :],
                                  last[:, lo:hi, :])
    nc.finalize()
    return nc


def _prep_inputs(input, weight, bias):
    inp = np.ascontiguousarray(input, dtype=np.float32)
    bis = np.ascontiguousarray(bias, dtype=np.float32)

    # [h, w, c, b], pre-scaled so fp8(32w) @ bf16(x/32) = w @ x exactly
    in2 = np.ascontiguousarray((inp / 32.0).transpose(2, 3, 1, 0))
    w8 = (np.asarray(weight, dtype=np.float32) * 32.0).astype(
        ml_dtypes.float8_e3m4)

    in_maps = []
    for core in range(NCORES):
        h0 = core * RPC
        img = np.zeros((64, RPC + 2, W, B), ml_dtypes.bfloat16)
        for hp in range(RPC + 2):
            h = h0 - 1 + hp
            if 0 <= h < H:
                img[:, hp, :, :] = in2[h].transpose(1, 0, 2)
        # [loc, O, C, ki, kj] -> [g][c][j, ki, kj, o]
        wc = w8[h0:h0 + RPC].reshape(LOCS, O, C, KK, KK)
        wt = np.ascontiguousarray(
            wc.transpose(2, 0, 3, 4, 1)          # [c, loc, ki, kj, o]
              .reshape(C, NG, G, KK, KK, O)
              .transpose(1, 0, 2, 3, 4, 5))      # [g, c, j, ki, kj, o]
        in_maps.append({
            "slab": img,
            "wt": wt,
            "bias": np.ascontiguousarray(
                bis.reshape(O, HO, WO)[:, h0:h0 + RPC, :]
                .reshape(O, LOCS)).astype(ml_dtypes.bfloat16),
        })
    return in_maps


_RUN_KW = {}  # test.py can inject trace=True etc.
_LAST_RESULT = [None]
_NC_CACHE = [None]


def kernel(input, weight, bias):
    from concourse.bass_utils import run_bass_kernel_spmd

    in_maps = _prep_inputs(input, weight, bias)
    if _NC_CACHE[0] is None:
        _NC_CACHE[0] = _build_bass()
    nc = _NC_CACHE[0]
    res = run_bass_kernel_spmd(nc, in_maps, core_ids=list(range(NCORES)),
                               **_RUN_KW)
    _LAST_RESULT[0] = res
    arr = np.stack([np.asarray(r["out"], dtype=np.float32)
                    for r in res.results])            # [core,hol,o,wo,b]
    out = arr.transpose(4, 2, 0, 1, 3).reshape(B, O, HO, WO)
    return np.ascontiguousarray(out)


# revision 24
# speedup vs baseline: 1.0390x; 1.0390x over previous
"""Local2d (unshared-weight conv) Bass kernel for 8 trn2 NeuronCores.

Problem: input (64,64,32,32), weight (32,32,128,64,3,3), bias (128,32,32)
-> out (64,128,32,32).  K=3, stride 1, pad 1.

Sharding: spatial over h_out — core i handles output rows 4i..4i+3 and
reads the disjoint weight slice for those rows, plus a 6-row input halo
slab.

The kernel is DMA-bound on the weight stream, so precision is chosen to
minimize bytes within the 2e-2 tolerance:
  - weights: fp8 e3m4, pre-scaled by 32 on host (keeps the randn/24
    values out of the subnormal range); 9.4MB/core.
  - input: fp8 e3m4, pre-scaled by 2; 0.8MB/core.
  - output: bf16 on device, upcast to f32 on host; 2.1MB/core.
The 64x product scale is folded into the per-location bias/descale op on
the scalar engine: out = Identity(psum/64 + bias).
Measured end-to-end rel err ~1.3e-2 vs the 2e-2 gate.

Per output location (ho,wo) the contraction (c,ki,kj)=576 is 9 PE
matmuls of K=64 (channels) accumulating in PSUM, moving operand = input
columns [64(c), 64(b)], stationary = per-location weights [64(c), 128(o)].
"""

import numpy as np
import ml_dtypes

B, C, O, KK, H, W = 64, 64, 128, 3, 32, 32
HO = WO = 32
NCORES = 8
RPC = HO // NCORES          # output rows per core
LOCS = RPC * WO             # locations per core
G = 8                       # locations per weight-DMA group
NG = LOCS // G


def _build_bass():
    from concourse import bacc
    import concourse.mybir as mybir
    from concourse.tile import TileContext

    f32 = mybir.dt.float32
    bf16 = mybir.dt.bfloat16
    f8 = mybir.dt.float8e3
    nc = bacc.Bacc("TRN2", target_bir_lowering=False, debug=False,
                   num_devices=NCORES)

    # input slab without the zero pad columns (those are memset on-chip):
    # [64(c), 6, 32, 64(b)] e3m4, values pre-scaled by 2.
    slab_d = nc.dram_tensor("slab", (64, RPC + 2, W, B), f8,
                            kind="ExternalInput").ap()
    # weights pre-scaled by 32, e3m4, partition-major per group:
    # [g][c(64 part)][j, ki, kj, o] fully contiguous per-partition runs.
    wt_d = nc.dram_tensor("wt", (NG, 64, G, KK, KK, O), f8,
                          kind="ExternalInput").ap()
    bias_d = nc.dram_tensor("bias", (O, LOCS), f32,
                            kind="ExternalInput").ap()
    out_d = nc.dram_tensor("out", (RPC, O, WO, B), bf16,
                           kind="ExternalOutput").ap()

    with TileContext(nc) as tc:
        with tc.tile_pool(name="xslab", bufs=1) as xpool, \
             tc.tile_pool(name="wpool", bufs=6) as wpool, \
             tc.tile_pool(name="bpool", bufs=1) as bpool, \
             tc.tile_pool(name="opool", bufs=4) as opool, \
             tc.tile_pool(name="psum", bufs=8, space="PSUM") as pspool:

            X = xpool.tile([64, RPC + 2, W + 2, B], f8)
            # pad columns 0 and 33 are zeros, built on-chip
            nc.vector.memset(X[:, :, 0:1, :], 0.0)
            nc.vector.memset(X[:, :, W + 1:W + 2, :], 0.0)
            # chunked so row-0 matmuls can start long before the full slab
            # lands: rows 0-2 cols 1-8 first (enough for the first locs),
            # the rest of rows 0-2 trickled between early weight chunks
            # (scalar queue, ahead of all activation ops), and rows 3-5
            # staged below on the Pool queue, just ahead of first use.
            nc.scalar.dma_start(X[:, 0:3, 1:9], slab_d[:, 0:3, 0:8])

            bias_t = bpool.tile([128, LOCS], f32)
            nc.gpsimd.dma_start(bias_t, bias_d)

            out_rows = {}
            for g in range(NG):
                wt = wpool.tile([64, G, KK, KK, O], f8, tag="wt")
                if g == 0:
                    # small first chunk so matmuls can start sooner
                    nc.sync.dma_start(wt[:, 0:2], wt_d[g, :, 0:2])
                    nc.sync.dma_start(wt[:, 2:G], wt_d[g, :, 2:G])
                else:
                    nc.sync.dma_start(wt, wt_d[g])
                if g == 0:
                    nc.scalar.dma_start(X[:, 0:3, 9:21], slab_d[:, 0:3, 8:20])
                elif g == 1:
                    nc.scalar.dma_start(X[:, 0:3, 21:33], slab_d[:, 0:3, 20:32])
                elif g in (3, 6, 10):
                    # stage slab rows just ahead of first use:
                    # row 3 from loc 32 (hol=1), row 4 from 64, row 5 from 96
                    r = 3 + (g > 3) + (g > 6)
                    nc.gpsimd.dma_start(X[:, r:r + 1, 1:33], slab_d[:, r:r + 1])

                for j in range(G):
                    loc = g * G + j
                    hol, wo = divmod(loc, WO)
                    if wo == 0:
                        out_rows[hol] = opool.tile([128, WO, B], bf16,
                                                   tag="orow",
                                                   name=f"orow{hol}")
                    orow = out_rows[hol]

                    ps = pspool.tile([128, B], f32, tag="ps", name=f"ps{loc}")
                    n = 0
                    for ki in range(KK):
                        for kj in range(KK):
                            nc.tensor.matmul(ps, wt[:, j, ki, kj, :],
                                             X[:, hol + ki, wo + kj, :],
                                             start=(n == 0), stop=(n == 8))
                            n += 1
                    # descale the 32x2 operand pre-scales and add bias, on
                    # the otherwise idle scalar engine: out = ps/64 + bias
                    nc.scalar.activation(
                        orow[:, wo, :], ps,
                        mybir.ActivationFunctionType.Identity,
                        bias=bias_t[:, loc, None], scale=1.0 / 64.0)

                    # rows 0-2: whole-row flushes on the Pool queue — their
                    # transfers slot into DMA idle gaps without ever blocking
                    # weight prefetch on the sync queue.
                    if hol != RPC - 1 and wo == WO - 1:
                        nc.gpsimd.dma_start(out_d[hol], orow)

            # last row: fine-grained flushes emitted after every weight DMA
            # so the in-order sync queue can never stall a weight load, and
            # the tail after the final matmul is one small chunk.
            last = out_rows[RPC - 1]
            for lo, hi in ((0, 8), (8, 16), (16, 24), (24, 28), (28, 32)):
                nc.sync.dma_start(out_d[RPC - 1, :, lo:hi, :],
                                  last[:, lo:hi, :])
    nc.finalize()
    return nc


def _prep_inputs(input, weight, bias):
    inp = np.ascontiguousarray(input, dtype=np.float32)
    bis = np.ascontiguousarray(bias, dtype=np.float32)

    # [h, w, c, b]; input x2 and weight x32 keep e3m4 quantization out of
    # the subnormal range; the 64x product scale is descaled on-device.
    in2 = np.ascontiguousarray((inp * 2.0).transpose(2, 3, 1, 0)).astype(
        ml_dtypes.float8_e3m4)
    w8 = (np.asarray(weight, dtype=np.float32) * 32.0).astype(
        ml_dtypes.float8_e3m4)

    in_maps = []
    for core in range(NCORES):
        h0 = core * RPC
        img = np.zeros((64, RPC + 2, W, B), ml_dtypes.float8_e3m4)
        for hp in range(RPC + 2):
            h = h0 - 1 + hp
            if 0 <= h < H:
                img[:, hp, :, :] = in2[h].transpose(1, 0, 2)
        # [loc, O, C, ki, kj] -> [g][c][j, ki, kj, o]
        wc = w8[h0:h0 + RPC].reshape(LOCS, O, C, KK, KK)
        wt = np.ascontiguousarray(
            wc.transpose(2, 0, 3, 4, 1)          # [c, loc, ki, kj, o]
              .reshape(C, NG, G, KK, KK, O)
              .transpose(1, 0, 2, 3, 4, 5))      # [g, c, j, ki, kj, o]
        in_maps.append({
            "slab": img,
            "wt": wt,
            "bias": np.ascontiguousarray(
                bis.reshape(O, HO, WO)[:, h0:h0 + RPC, :].reshape(O, LOCS)),
        })
    return in_maps


_RUN_KW = {}  # test.py can inject trace=True etc.
_LAST_RESULT = [None]
_NC_CACHE = [None]


def kernel(input, weight, bias):
    from concourse.bass_utils import run_bass_kernel_spmd

    in_maps = _prep_inputs(input, weight, bias)
    if _NC_CACHE[0] is None:
        _NC_CACHE[0] = _build_bass()
    nc = _NC_CACHE[0]
    res = run_bass_kernel_spmd(nc, in_maps, core_ids=list(range(NCORES)),
                               **_RUN_KW)
    _LAST_RESULT[0] = res
    arr = np.stack([np.asarray(r["out"], dtype=np.float32)
                    for r in res.results])            # [core,hol,o,wo,b]
    out = arr.transpose(4, 2, 0, 1, 3).reshape(B, O, HO, WO)
    return np.ascontiguousarray(out)
